# revision 12
# baseline (speedup 1.0000x reference)
"""ANFIS kernel for 8 TRN2 NeuronCores — pure batch data-parallel.

Math: out[b,o] = prod_f(x[b,f]) * w[b,o]^32 where
  w = sum_r(p_r * m_r) / sum_r(m_r),  m_r = exp(-((y-c_r)/s_r)^2),
  y = MLP(x).  exp(-z^2) is computed on the ScalarEngine as
  Derivative_Erf(scale*y + bias) (= 2/sqrt(pi) * exp(-z^2); the constant
  cancels in the normalization).  D = sum m and N = sum p*m are reduced
  over rules with fp16 TensorE matmuls (identity / diag(p) stationary),
  accumulating in f32 PSUM.
"""
import sys

if "/opt/trn_rl_repo" not in sys.path:
    sys.path.insert(0, "/opt/trn_rl_repo")

import numpy as np
import ml_dtypes
ml_bf16 = ml_dtypes.bfloat16

import concourse.bacc as bacc
import concourse.mybir as mybir
from concourse.bass_utils import run_bass_kernel_spmd
from concourse.tile import TileContext
from concourse.mybir import AluOpType as Op

B, IN_DIM, OUT_DIM, N_RULES, H = 8192, 32, 256, 16, 256
N_CORES = 8
BL = B // N_CORES          # 1024 batch rows per core
P = 128                    # partitions
NOT = OUT_DIM // P         # 2 o-tiles
NJ = H // P                # 2 hidden j-tiles
FD = 512                   # matmul free-dim chunk (one PSUM bank)
F32 = mybir.dt.float32
F16 = mybir.dt.float16

# packed f32 constant columns: b1t | b2t | scl | bia | eyef | xbp
C_B1 = 0
C_B2 = C_B1 + NJ
C_SCL = C_B2 + NJ
C_BIA = C_SCL + NOT * N_RULES
C_EYE = C_BIA + NOT * N_RULES
C_XBP = C_EYE + P
C_END = C_XBP + (BL // P) * IN_DIM

_nc_cache = None


def _build():
    global _nc_cache
    if _nc_cache is not None:
        return _nc_cache
    nc = bacc.Bacc(None, target_bir_lowering=False, debug=False, num_devices=N_CORES)

    xw_d = nc.declare_dram_parameter("xw", [3 * IN_DIM, BL], mybir.dt.bfloat16, isOutput=False)
    w1s_d = nc.declare_dram_parameter("w1s", [3 * IN_DIM, H], mybir.dt.bfloat16, isOutput=False)
    cst_d = nc.declare_dram_parameter("cst", [P, C_END], F32, isOutput=False)
    w23_d = nc.declare_dram_parameter("w23", [P, (NJ * NJ + NJ * NOT) * P], F16, isOutput=False)
    f16c_d = nc.declare_dram_parameter("f16c", [P, P + NOT * N_RULES * P], F16, isOutput=False)
    ones1_d = nc.declare_dram_parameter("ones1", [1, P], F32, isOutput=False)
    out_d = nc.declare_dram_parameter("out", [OUT_DIM, BL], F32, isOutput=True)

    DERF = mybir.ActivationFunctionType.Derivative_Erf
    SQ = mybir.ActivationFunctionType.Square
    RELU = mybir.ActivationFunctionType.Relu
    NCH = BL // FD  # chunks

    with TileContext(nc) as tc:
        with tc.sbuf_pool(name="sb", bufs=1) as sb:
            # ---- warm the PE clock gate on garbage data during the DMA wait ----
            junk16 = sb.tile([P, P + 256], F16)
            nc.vector.memset(junk16[:], 0.0)
            with tc.psum_pool(name="ps_warm", bufs=1) as ps_warm:
                wt = ps_warm.tile([P, 256], F32, tag="warm")
                for _ in range(8):
                    nc.tensor.matmul(wt[:], junk16[:, :P], junk16[:, P:], start=True, stop=True)

            # ---- loads: xw/w1s first so L1 starts ASAP ----
            xw = sb.tile([3 * IN_DIM, BL], mybir.dt.bfloat16)
            nc.sync.dma_start(out=xw[:], in_=xw_d[:])
            w1s = sb.tile([3 * IN_DIM, H], mybir.dt.bfloat16)
            nc.sync.dma_start(out=w1s[:], in_=w1s_d[:])
            cst = sb.tile([P, C_END], F32)
            nc.sync.dma_start(out=cst[:], in_=cst_d[:])
            w23 = sb.tile([P, (NJ * NJ + NJ * NOT) * P], F16)
            nc.sync.dma_start(out=w23[:], in_=w23_d[:])
            f16c = sb.tile([P, P + NOT * N_RULES * P], F16)
            nc.sync.dma_start(out=f16c[:], in_=f16c_d[:])
            ones1 = sb.tile([1, P], F32)
            nc.sync.dma_start(out=ones1[:], in_=ones1_d[:])

            b1t = cst[:, C_B1:C_B1 + NJ]
            b2t = cst[:, C_B2:C_B2 + NJ]
            scl = cst[:, C_SCL:C_SCL + NOT * N_RULES]
            bia = cst[:, C_BIA:C_BIA + NOT * N_RULES]
            eyef = cst[:, C_EYE:C_EYE + P]
            xbp = cst[:, C_XBP:C_END]
            eye16 = f16c[:, :P]
            dgs = f16c[:, P:]

            def W2blk(k, j):
                return w23[:, (k * NJ + j) * P:(k * NJ + j + 1) * P]

            def W3blk(k, j):
                off = NJ * NJ * P
                return w23[:, off + (k * NOT + j) * P:off + (k * NOT + j + 1) * P]

            # ---- P[b] = prod_f x[b,f], replicated across partitions ----
            # P_all[p, t] = P(b = 128*t + p); DMA-permute into one row, then
            # gpsimd partition-broadcast to [128, BL]. No PSUM, no TensorE.
            P_rep = sb.tile([P, BL], F32)
            P_all = sb.tile([P, BL // P], F32)
            nc.vector.tensor_reduce(
                P_all[:],
                xbp.rearrange("p (t f) -> p t f", f=IN_DIM),
                mybir.AxisListType.X, Op.mult,
            )
            P_row = sb.tile([1, BL], F32)
            for t in range(BL // P):
                nc.sync.dma_start(out=P_row[0:1, t * P:(t + 1) * P], in_=P_all[:, t:t + 1])
            nc.gpsimd.partition_broadcast(P_rep[:], P_row[0:1, :])

            def relu_bias(dst, src_psum, bias_col, j):
                # chunk-split across ACT and DVE to cut inter-layer latency
                for c in range(NCH):
                    cs = slice(c * FD, (c + 1) * FD)
                    if (c + j) % 2 == 0:
                        nc.scalar.activation(dst[cs_p(cs)], src_psum[cs_p(cs)], RELU,
                                             bias=bias_col, scale=1.0)
                    else:
                        nc.vector.tensor_scalar(dst[cs_p(cs)], src_psum[cs_p(cs)],
                                                bias_col, 0.0, Op.add, Op.max)

            def cs_p(cs):
                return (slice(None), cs)

            hT = []
            h2T = []
            with tc.psum_pool(name="ps_y", bufs=2) as ps_y:
                yT = []
                with tc.psum_pool(name="ps_mlp", bufs=2) as ps_mlp:
                    for j in range(NJ):
                        l1 = ps_mlp.tile([P, BL], F32, tag="mlp")
                        for c in range(NCH):
                            nc.tensor.matmul(
                                l1[:, c * FD:(c + 1) * FD],
                                w1s[:, j * P:(j + 1) * P],
                                xw[:, c * FD:(c + 1) * FD],
                                start=True, stop=True,
                            )
                        h = sb.tile([P, BL], F16, name=f"hT{j}")
                        relu_bias(h[:], l1[:], b1t[:, j:j + 1], j)
                        hT.append(h)
                    for j in range(NJ):
                        l2 = ps_mlp.tile([P, BL], F32, tag="mlp")
                        for c in range(NCH):
                            for k in range(NJ):
                                nc.tensor.matmul(
                                    l2[:, c * FD:(c + 1) * FD],
                                    W2blk(k, j),
                                    hT[k][:, c * FD:(c + 1) * FD],
                                    start=(k == 0), stop=(k == NJ - 1),
                                )
                        h = sb.tile([P, BL], F16, name=f"h2T{j}")
                        relu_bias(h[:], l2[:], b2t[:, j:j + 1], j)
                        h2T.append(h)
                    for j in range(NOT):
                        l3 = ps_y.tile([P, BL], F32, tag="yt")
                        for c in range(NCH):
                            for k in range(NJ):
                                nc.tensor.matmul(
                                    l3[:, c * FD:(c + 1) * FD],
                                    W3blk(k, j),
                                    h2T[k][:, c * FD:(c + 1) * FD],
                                    start=(k == 0), stop=(k == NJ - 1),
                                )
                        yT.append(l3)

                # ---- memberships + D/N + w per o-tile ----
                pad = sb.tile([P, 512], F32, name="pad")
                nc.vector.memset(pad[:, 0:8], 0.0)
                with tc.psum_pool(name="ps_dn", bufs=1) as ps_dn:
                    for ot in range(NOT):
                        D = ps_dn.tile([P, BL], F32, tag="D", name=f"D{ot}")
                        N = ps_dn.tile([P, BL], F32, tag="N", name=f"N{ot}")
                        for r in range(N_RULES):
                            idx = ot * N_RULES + r
                            m = sb.tile([P, BL], F16, tag="m", bufs=4, name=f"m{idx}")
                            if ot == NOT - 1 and r == N_RULES - 1:
                                for c in range(NCH):
                                    cs = slice(c * FD, (c + 1) * FD)
                                    nc.scalar.activation(
                                        m[:, cs], yT[ot][:, cs], DERF,
                                        bias=bia[:, idx:idx + 1], scale=scl[:, idx:idx + 1],
                                    )
                            else:
                                nc.scalar.activation(
                                    m[:], yT[ot][:], DERF,
                                    bias=bia[:, idx:idx + 1], scale=scl[:, idx:idx + 1],
                                )
                            for c in range(NCH):
                                cs = slice(c * FD, (c + 1) * FD)
                                nc.tensor.matmul(D[:, cs], eye16, m[:, cs],
                                                 start=(r == 0), stop=(r == N_RULES - 1))
                                nc.tensor.matmul(N[:, cs], dgs[:, idx * P:(idx + 1) * P], m[:, cs],
                                                 start=(r == 0), stop=(r == N_RULES - 1))
                        rD = sb.tile([P, BL], F32, tag="rD", bufs=2, name=f"rD{ot}")
                        nc.vector.reciprocal_approx_fast(rD[:], D[:])
                        w = sb.tile([P, BL], F32, tag="w", bufs=2, name=f"w{ot}")
                        nc.vector.tensor_tensor(w[:], N[:], rD[:], Op.mult)
                        o = sb.tile([P, BL], F32, tag="osb", bufs=2, name=f"osb{ot}")
                        if ot < NOT - 1:
                            for _ in range(5):
                                nc.vector.tensor_tensor(w[:], w[:], w[:], Op.mult)
                            nc.vector.tensor_tensor(o[:], w[:], P_rep[:], Op.mult)
                            nc.sync.dma_start(out=out_d[ot * P:(ot + 1) * P, :], in_=o[:])
                        else:
                            # last o-tile = serial tail: w^32 = exp(16*ln(w^2))
                            # on ACT (Square/Ln/Exp share the natural_log_exp set;
                            # the table switch hides under recip/w on DVE)
                            w2 = sb.tile([P, BL], F32, name="w2tail")
                            nc.scalar.activation(w2[:], w[:], SQ)
                            nc.scalar.activation(w2[:], w2[:], mybir.ActivationFunctionType.Ln)
                            nc.scalar.activation(w2[:], w2[:], mybir.ActivationFunctionType.Exp,
                                                 scale=16.0)
                            h0 = slice(0, BL // 2)
                            h1 = slice(BL // 2, BL)
                            nc.vector.tensor_tensor(o[:, h0], w2[:, h0], P_rep[:, h0], Op.mult)
                            nc.sync.dma_start(out=out_d[ot * P:(ot + 1) * P, :BL // 2], in_=o[:, h0])
                            nc.vector.tensor_tensor(o[:, h1], w2[:, h1], P_rep[:, h1], Op.mult)
                            nc.sync.dma_start(out=out_d[ot * P:(ot + 1) * P, BL // 2:], in_=o[:, h1])

    nc.finalize()
    _nc_cache = nc
    return nc


def _prepare_in_maps(x, W1, b1, W2, b2, W3, b3, centers, widths, params):
    x = np.ascontiguousarray(x, dtype=np.float32)
    W1 = np.asarray(W1, np.float32); b1 = np.asarray(b1, np.float32)
    W2 = np.asarray(W2, np.float32); b2 = np.asarray(b2, np.float32)
    W3 = np.asarray(W3, np.float32); b3 = np.asarray(b3, np.float32)
    centers = np.asarray(centers, np.float32)
    widths = np.asarray(widths, np.float32)
    params = np.asarray(params, np.float32)

    def pack_w(W, nj_out):
        blocks = []
        for k in range(W.shape[0] // P):
            for j in range(nj_out):
                blocks.append(W[k * P:(k + 1) * P, j * P:(j + 1) * P])
        return np.concatenate(blocks, axis=1)

    w23 = np.ascontiguousarray(
        np.concatenate([pack_w(W2, NJ), pack_w(W3, NOT)], axis=1).astype(np.float16))

    b1t = b1.reshape(NJ, P).T
    b2t = b2.reshape(NJ, P).T
    inv = (1.0 / widths).astype(np.float32)
    biasf = ((b3[:, None] - centers) * inv).astype(np.float32)
    scl = inv.reshape(NOT, P, N_RULES).transpose(1, 0, 2).reshape(P, NOT * N_RULES)
    bia = biasf.reshape(NOT, P, N_RULES).transpose(1, 0, 2).reshape(P, NOT * N_RULES)
    eyef = np.eye(P, dtype=np.float32)

    ph = params.astype(np.float16)
    dgs = np.zeros((P, NOT * N_RULES * P), np.float16)
    for ot in range(NOT):
        for r in range(N_RULES):
            idx = ot * N_RULES + r
            dgs[:, idx * P:(idx + 1) * P] = np.diag(ph[ot * P:(ot + 1) * P, r])
    f16c = np.ascontiguousarray(np.concatenate([np.eye(P, dtype=np.float16), dgs], axis=1))

    ones1 = np.ones((1, P), np.float32)

    # L1 bf16 hi/lo stacking: y1 = W1h.T@xh + W1l.T@xh + W1h.T@xl
    W1h = W1.astype(ml_bf16)
    W1l = (W1 - W1h.astype(np.float32)).astype(ml_bf16)
    w1s = np.ascontiguousarray(np.concatenate([W1h, W1l, W1h], axis=0))  # [96, H]

    in_maps = []
    for i in range(N_CORES):
        xs = x[i * BL:(i + 1) * BL]                              # [BL, 32]
        xT = np.ascontiguousarray(xs.T)                          # [32, BL]
        xh = xT.astype(ml_bf16)
        xl = (xT - xh.astype(np.float32)).astype(ml_bf16)
        xw = np.ascontiguousarray(np.concatenate([xh, xh, xl], axis=0))  # [96, BL]
        xbp = xs.reshape(BL // P, P, IN_DIM).transpose(1, 0, 2).reshape(P, -1)
        cst = np.ascontiguousarray(
            np.concatenate([b1t, b2t, scl, bia, eyef, xbp], axis=1))
        in_maps.append(dict(xw=xw, w1s=w1s, cst=cst, w23=w23, f16c=f16c, ones1=ones1))
    return in_maps


def run(trace=False, **inputs):
    nc = _build()
    in_maps = _prepare_in_maps(**inputs)
    res = run_bass_kernel_spmd(nc, in_maps, core_ids=list(range(N_CORES)), trace=trace)
    outs = [res.results[i]["out"].T for i in range(N_CORES)]     # each [BL, O]
    full = np.ascontiguousarray(np.concatenate(outs, axis=0), dtype=np.float32)
    return full, res


def kernel(**inputs) -> np.ndarray:
    full, _ = run(trace=False, **inputs)
    return full


# revision 13
# speedup vs baseline: 1.0185x; 1.0185x over previous
"""ANFIS kernel for 8 TRN2 NeuronCores — pure batch data-parallel.

Math: out[b,o] = prod_f(x[b,f]) * w[b,o]^32 where
  w = sum_r(p_r * m_r) / sum_r(m_r),  m_r = exp(-((y-c_r)/s_r)^2),
  y = MLP(x).  exp(-z^2) is computed on the ScalarEngine as
  Derivative_Erf(scale*y + bias) (= 2/sqrt(pi) * exp(-z^2); the constant
  cancels in the normalization).  D = sum m and N = sum p*m are reduced
  over rules with fp16 TensorE matmuls (identity / diag(p) stationary),
  accumulating in f32 PSUM.
"""
import sys

if "/opt/trn_rl_repo" not in sys.path:
    sys.path.insert(0, "/opt/trn_rl_repo")

import numpy as np
import ml_dtypes
ml_bf16 = ml_dtypes.bfloat16

import concourse.bacc as bacc
import concourse.mybir as mybir
from concourse.bass_utils import run_bass_kernel_spmd
from concourse.tile import TileContext
from concourse.mybir import AluOpType as Op

B, IN_DIM, OUT_DIM, N_RULES, H = 8192, 32, 256, 16, 256
N_CORES = 8
BL = B // N_CORES          # 1024 batch rows per core
P = 128                    # partitions
NOT = OUT_DIM // P         # 2 o-tiles
NJ = H // P                # 2 hidden j-tiles
FD = 512                   # matmul free-dim chunk (one PSUM bank)
F32 = mybir.dt.float32
F16 = mybir.dt.float16

# packed f32 constant columns: b1t | b2t | scl | bia | eyef | xbp
C_B1 = 0
C_B2 = C_B1 + NJ
C_SCL = C_B2 + NJ
C_BIA = C_SCL + NOT * N_RULES
C_EYE = C_BIA + NOT * N_RULES
C_XBP = C_EYE + P
C_END = C_XBP + (BL // P) * IN_DIM

_nc_cache = None


def _build():
    global _nc_cache
    if _nc_cache is not None:
        return _nc_cache
    nc = bacc.Bacc(None, target_bir_lowering=False, debug=False, num_devices=N_CORES)

    xw_d = nc.declare_dram_parameter("xw", [3 * IN_DIM, BL], mybir.dt.bfloat16, isOutput=False)
    w1s_d = nc.declare_dram_parameter("w1s", [3 * IN_DIM, H], mybir.dt.bfloat16, isOutput=False)
    cst_d = nc.declare_dram_parameter("cst", [P, C_END], F32, isOutput=False)
    w23_d = nc.declare_dram_parameter("w23", [P, (NJ * NJ + NJ * NOT) * P], F16, isOutput=False)
    f16c_d = nc.declare_dram_parameter("f16c", [P, P + NOT * N_RULES * P], F16, isOutput=False)
    ones1_d = nc.declare_dram_parameter("ones1", [1, P], F32, isOutput=False)
    out_d = nc.declare_dram_parameter("out", [OUT_DIM, BL], F32, isOutput=True)

    DERF = mybir.ActivationFunctionType.Derivative_Erf
    SQ = mybir.ActivationFunctionType.Square
    RELU = mybir.ActivationFunctionType.Relu
    NCH = BL // FD  # chunks

    with TileContext(nc) as tc:
        with tc.sbuf_pool(name="sb", bufs=1) as sb:
            # ---- warm the PE clock gate on garbage data during the DMA wait ----
            junk16 = sb.tile([P, P + 256], F16)
            nc.vector.memset(junk16[:], 0.0)
            with tc.psum_pool(name="ps_warm", bufs=1) as ps_warm:
                wt = ps_warm.tile([P, 256], F32, tag="warm")
                for _ in range(8):
                    nc.tensor.matmul(wt[:], junk16[:, :P], junk16[:, P:], start=True, stop=True)

            # ---- loads: xw/w1s first so L1 starts ASAP ----
            xw = sb.tile([3 * IN_DIM, BL], mybir.dt.bfloat16)
            nc.sync.dma_start(out=xw[:], in_=xw_d[:])
            w1s = sb.tile([3 * IN_DIM, H], mybir.dt.bfloat16)
            nc.sync.dma_start(out=w1s[:], in_=w1s_d[:])
            cst = sb.tile([P, C_END], F32)
            nc.sync.dma_start(out=cst[:], in_=cst_d[:])
            w23 = sb.tile([P, (NJ * NJ + NJ * NOT) * P], F16)
            nc.sync.dma_start(out=w23[:], in_=w23_d[:])
            f16c = sb.tile([P, P + NOT * N_RULES * P], F16)
            nc.sync.dma_start(out=f16c[:], in_=f16c_d[:])
            ones1 = sb.tile([1, P], F32)
            nc.sync.dma_start(out=ones1[:], in_=ones1_d[:])

            b1t = cst[:, C_B1:C_B1 + NJ]
            b2t = cst[:, C_B2:C_B2 + NJ]
            scl = cst[:, C_SCL:C_SCL + NOT * N_RULES]
            bia = cst[:, C_BIA:C_BIA + NOT * N_RULES]
            eyef = cst[:, C_EYE:C_EYE + P]
            xbp = cst[:, C_XBP:C_END]
            eye16 = f16c[:, :P]
            dgs = f16c[:, P:]

            def W2blk(k, j):
                return w23[:, (k * NJ + j) * P:(k * NJ + j + 1) * P]

            def W3blk(k, j):
                off = NJ * NJ * P
                return w23[:, off + (k * NOT + j) * P:off + (k * NOT + j + 1) * P]

            # ---- P[b] = prod_f x[b,f], replicated across partitions ----
            # P_all[p, t] = P(b = 128*t + p); DMA-permute into one row, then
            # gpsimd partition-broadcast to [128, BL]. No PSUM, no TensorE.
            P_rep = sb.tile([P, BL], F32)
            P_all = sb.tile([P, BL // P], F32)
            nc.vector.tensor_reduce(
                P_all[:],
                xbp.rearrange("p (t f) -> p t f", f=IN_DIM),
                mybir.AxisListType.X, Op.mult,
            )
            P_row = sb.tile([1, BL], F32)
            for t in range(BL // P):
                nc.sync.dma_start(out=P_row[0:1, t * P:(t + 1) * P], in_=P_all[:, t:t + 1])
            nc.gpsimd.partition_broadcast(P_rep[:], P_row[0:1, :])

            def relu_bias(dst, src_psum, bias_col, j):
                # chunk-split across ACT and DVE to cut inter-layer latency
                for c in range(NCH):
                    cs = slice(c * FD, (c + 1) * FD)
                    if (c + j) % 2 == 0:
                        nc.scalar.activation(dst[cs_p(cs)], src_psum[cs_p(cs)], RELU,
                                             bias=bias_col, scale=1.0)
                    else:
                        nc.vector.tensor_scalar(dst[cs_p(cs)], src_psum[cs_p(cs)],
                                                bias_col, 0.0, Op.add, Op.max)

            def cs_p(cs):
                return (slice(None), cs)

            hT = []
            h2T = []
            with tc.psum_pool(name="ps_y", bufs=2) as ps_y:
                yT = []
                with tc.psum_pool(name="ps_mlp", bufs=2) as ps_mlp:
                    for j in range(NJ):
                        l1 = ps_mlp.tile([P, BL], F32, tag="mlp")
                        for c in range(NCH):
                            nc.tensor.matmul(
                                l1[:, c * FD:(c + 1) * FD],
                                w1s[:, j * P:(j + 1) * P],
                                xw[:, c * FD:(c + 1) * FD],
                                start=True, stop=True,
                            )
                        h = sb.tile([P, BL], F16, name=f"hT{j}")
                        relu_bias(h[:], l1[:], b1t[:, j:j + 1], j)
                        hT.append(h)
                    for j in range(NJ):
                        l2 = ps_mlp.tile([P, BL], F32, tag="mlp")
                        for c in range(NCH):
                            for k in range(NJ):
                                nc.tensor.matmul(
                                    l2[:, c * FD:(c + 1) * FD],
                                    W2blk(k, j),
                                    hT[k][:, c * FD:(c + 1) * FD],
                                    start=(k == 0), stop=(k == NJ - 1),
                                )
                        h = sb.tile([P, BL], F16, name=f"h2T{j}")
                        relu_bias(h[:], l2[:], b2t[:, j:j + 1], j)
                        h2T.append(h)
                    for j in range(NOT):
                        l3 = ps_y.tile([P, BL], F32, tag="yt")
                        for c in range(NCH):
                            for k in range(NJ):
                                nc.tensor.matmul(
                                    l3[:, c * FD:(c + 1) * FD],
                                    W3blk(k, j),
                                    h2T[k][:, c * FD:(c + 1) * FD],
                                    start=(k == 0), stop=(k == NJ - 1),
                                )
                        yT.append(l3)

                # ---- memberships + D/N + w per o-tile ----
                pad = sb.tile([P, 512], F32, name="pad")
                nc.vector.memset(pad[:, 0:8], 0.0)
                with tc.psum_pool(name="ps_dn", bufs=1) as ps_dn:
                    for ot in range(NOT):
                        D = ps_dn.tile([P, BL], F32, tag="D", name=f"D{ot}")
                        N = ps_dn.tile([P, BL], F32, tag="N", name=f"N{ot}")
                        for r in range(N_RULES):
                            idx = ot * N_RULES + r
                            m = sb.tile([P, BL], F16, tag="m", bufs=4, name=f"m{idx}")
                            if ot == NOT - 1 and r == N_RULES - 1:
                                for c in range(NCH):
                                    cs = slice(c * FD, (c + 1) * FD)
                                    nc.scalar.activation(
                                        m[:, cs], yT[ot][:, cs], DERF,
                                        bias=bia[:, idx:idx + 1], scale=scl[:, idx:idx + 1],
                                    )
                            else:
                                nc.scalar.activation(
                                    m[:], yT[ot][:], DERF,
                                    bias=bia[:, idx:idx + 1], scale=scl[:, idx:idx + 1],
                                )
                            for c in range(NCH):
                                cs = slice(c * FD, (c + 1) * FD)
                                nc.tensor.matmul(D[:, cs], eye16, m[:, cs],
                                                 start=(r == 0), stop=(r == N_RULES - 1))
                                nc.tensor.matmul(N[:, cs], dgs[:, idx * P:(idx + 1) * P], m[:, cs],
                                                 start=(r == 0), stop=(r == N_RULES - 1))
                        rD = sb.tile([P, BL], F32, tag="rD", bufs=2, name=f"rD{ot}")
                        nc.vector.reciprocal_approx_fast(rD[:], D[:])
                        w = sb.tile([P, BL], F32, tag="w", bufs=2, name=f"w{ot}")
                        nc.vector.tensor_tensor(w[:], N[:], rD[:], Op.mult)
                        o = sb.tile([P, BL], F32, tag="osb", bufs=2, name=f"osb{ot}")
                        if ot < NOT - 1:
                            for _ in range(5):
                                nc.vector.tensor_tensor(w[:], w[:], w[:], Op.mult)
                            nc.vector.tensor_tensor(o[:], w[:], P_rep[:], Op.mult)
                            nc.sync.dma_start(out=out_d[ot * P:(ot + 1) * P, :], in_=o[:])
                        else:
                            # last o-tile = serial tail: pipeline halves on DVE + ACT
                            h0 = slice(0, BL // 2)
                            h1 = slice(BL // 2, BL)
                            for _ in range(5):
                                nc.vector.tensor_tensor(w[:, h0], w[:, h0], w[:, h0], Op.mult)
                                nc.scalar.activation(w[:, h1], w[:, h1], SQ)
                            nc.vector.tensor_tensor(o[:, h0], w[:, h0], P_rep[:, h0], Op.mult)
                            nc.sync.dma_start(out=out_d[ot * P:(ot + 1) * P, :BL // 2], in_=o[:, h0])
                            nc.vector.tensor_tensor(o[:, h1], w[:, h1], P_rep[:, h1], Op.mult)
                            nc.sync.dma_start(out=out_d[ot * P:(ot + 1) * P, BL // 2:], in_=o[:, h1])

    nc.finalize()
    _nc_cache = nc
    return nc


def _prepare_in_maps(x, W1, b1, W2, b2, W3, b3, centers, widths, params):
    x = np.ascontiguousarray(x, dtype=np.float32)
    W1 = np.asarray(W1, np.float32); b1 = np.asarray(b1, np.float32)
    W2 = np.asarray(W2, np.float32); b2 = np.asarray(b2, np.float32)
    W3 = np.asarray(W3, np.float32); b3 = np.asarray(b3, np.float32)
    centers = np.asarray(centers, np.float32)
    widths = np.asarray(widths, np.float32)
    params = np.asarray(params, np.float32)

    def pack_w(W, nj_out):
        blocks = []
        for k in range(W.shape[0] // P):
            for j in range(nj_out):
                blocks.append(W[k * P:(k + 1) * P, j * P:(j + 1) * P])
        return np.concatenate(blocks, axis=1)

    w23 = np.ascontiguousarray(
        np.concatenate([pack_w(W2, NJ), pack_w(W3, NOT)], axis=1).astype(np.float16))

    b1t = b1.reshape(NJ, P).T
    b2t = b2.reshape(NJ, P).T
    inv = (1.0 / widths).astype(np.float32)
    biasf = ((b3[:, None] - centers) * inv).astype(np.float32)
    scl = inv.reshape(NOT, P, N_RULES).transpose(1, 0, 2).reshape(P, NOT * N_RULES)
    bia = biasf.reshape(NOT, P, N_RULES).transpose(1, 0, 2).reshape(P, NOT * N_RULES)
    eyef = np.eye(P, dtype=np.float32)

    ph = params.astype(np.float16)
    dgs = np.zeros((P, NOT * N_RULES * P), np.float16)
    for ot in range(NOT):
        for r in range(N_RULES):
            idx = ot * N_RULES + r
            dgs[:, idx * P:(idx + 1) * P] = np.diag(ph[ot * P:(ot + 1) * P, r])
    f16c = np.ascontiguousarray(np.concatenate([np.eye(P, dtype=np.float16), dgs], axis=1))

    ones1 = np.ones((1, P), np.float32)

    # L1 bf16 hi/lo stacking: y1 = W1h.T@xh + W1l.T@xh + W1h.T@xl
    W1h = W1.astype(ml_bf16)
    W1l = (W1 - W1h.astype(np.float32)).astype(ml_bf16)
    w1s = np.ascontiguousarray(np.concatenate([W1h, W1l, W1h], axis=0))  # [96, H]

    in_maps = []
    for i in range(N_CORES):
        xs = x[i * BL:(i + 1) * BL]                              # [BL, 32]
        xT = np.ascontiguousarray(xs.T)                          # [32, BL]
        xh = xT.astype(ml_bf16)
        xl = (xT - xh.astype(np.float32)).astype(ml_bf16)
        xw = np.ascontiguousarray(np.concatenate([xh, xh, xl], axis=0))  # [96, BL]
        xbp = xs.reshape(BL // P, P, IN_DIM).transpose(1, 0, 2).reshape(P, -1)
        cst = np.ascontiguousarray(
            np.concatenate([b1t, b2t, scl, bia, eyef, xbp], axis=1))
        in_maps.append(dict(xw=xw, w1s=w1s, cst=cst, w23=w23, f16c=f16c, ones1=ones1))
    return in_maps


def run(trace=False, **inputs):
    nc = _build()
    in_maps = _prepare_in_maps(**inputs)
    res = run_bass_kernel_spmd(nc, in_maps, core_ids=list(range(N_CORES)), trace=trace)
    outs = [res.results[i]["out"].T for i in range(N_CORES)]     # each [BL, O]
    full = np.ascontiguousarray(np.concatenate(outs, axis=0), dtype=np.float32)
    return full, res


def kernel(**inputs) -> np.ndarray:
    full, _ = run(trace=False, **inputs)
    return full


# revision 14
# speedup vs baseline: 1.0336x; 1.0149x over previous
"""ANFIS kernel for 8 TRN2 NeuronCores — pure batch data-parallel.

Math: out[b,o] = prod_f(x[b,f]) * w[b,o]^32 where
  w = sum_r(p_r * m_r) / sum_r(m_r),  m_r = exp(-((y-c_r)/s_r)^2),
  y = MLP(x).  exp(-z^2) is computed on the ScalarEngine as
  Derivative_Erf(scale*y + bias) (= 2/sqrt(pi) * exp(-z^2); the constant
  cancels in the normalization).  D = sum m and N = sum p*m are reduced
  over rules with fp16 TensorE matmuls (identity / diag(p) stationary),
  accumulating in f32 PSUM.
"""
import sys

if "/opt/trn_rl_repo" not in sys.path:
    sys.path.insert(0, "/opt/trn_rl_repo")

import numpy as np
import ml_dtypes
ml_bf16 = ml_dtypes.bfloat16

import concourse.bacc as bacc
import concourse.mybir as mybir
from concourse.bass_utils import run_bass_kernel_spmd
from concourse.tile import TileContext
from concourse.mybir import AluOpType as Op

B, IN_DIM, OUT_DIM, N_RULES, H = 8192, 32, 256, 16, 256
N_CORES = 8
BL = B // N_CORES          # 1024 batch rows per core
P = 128                    # partitions
NOT = OUT_DIM // P         # 2 o-tiles
NJ = H // P                # 2 hidden j-tiles
FD = 512                   # matmul free-dim chunk (one PSUM bank)
F32 = mybir.dt.float32
F16 = mybir.dt.float16

# packed f32 constant columns: b1t | b2t | scl | bia | eyef | xbp
C_B1 = 0
C_B2 = C_B1 + NJ
C_SCL = C_B2 + NJ
C_BIA = C_SCL + NOT * N_RULES
C_EYE = C_BIA + NOT * N_RULES
C_XBP = C_EYE + P
C_END = C_XBP + (BL // P) * IN_DIM

_nc_cache = None


def _build():
    global _nc_cache
    if _nc_cache is not None:
        return _nc_cache
    nc = bacc.Bacc(None, target_bir_lowering=False, debug=False, num_devices=N_CORES)

    xw_d = nc.declare_dram_parameter("xw", [3 * IN_DIM, BL], mybir.dt.bfloat16, isOutput=False)
    w1s_d = nc.declare_dram_parameter("w1s", [3 * IN_DIM, H], mybir.dt.bfloat16, isOutput=False)
    cst_d = nc.declare_dram_parameter("cst", [P, C_END], F32, isOutput=False)
    w23_d = nc.declare_dram_parameter("w23", [P, (NJ * NJ + NJ * NOT) * P], F16, isOutput=False)
    f16c_d = nc.declare_dram_parameter("f16c", [P, P + NOT * N_RULES * P], F16, isOutput=False)
    ones1_d = nc.declare_dram_parameter("ones1", [1, P], F32, isOutput=False)
    out_d = nc.declare_dram_parameter("out", [OUT_DIM, BL], F32, isOutput=True)

    DERF = mybir.ActivationFunctionType.Derivative_Erf
    SQ = mybir.ActivationFunctionType.Square
    RELU = mybir.ActivationFunctionType.Relu
    NCH = BL // FD  # chunks

    with TileContext(nc) as tc:
        with tc.sbuf_pool(name="sb", bufs=1) as sb:
            # ---- warm the PE clock gate on garbage data during the DMA wait ----
            junk16 = sb.tile([P, P + 256], F16)
            nc.vector.memset(junk16[:], 0.0)
            with tc.psum_pool(name="ps_warm", bufs=1) as ps_warm:
                wt = ps_warm.tile([P, 256], F32, tag="warm")
                for _ in range(11):
                    nc.tensor.matmul(wt[:], junk16[:, :P], junk16[:, P:], start=True, stop=True)

            # ---- loads: xw/w1s first so L1 starts ASAP ----
            xw = sb.tile([3 * IN_DIM, BL], mybir.dt.bfloat16)
            nc.sync.dma_start(out=xw[:], in_=xw_d[:])
            w1s = sb.tile([3 * IN_DIM, H], mybir.dt.bfloat16)
            nc.sync.dma_start(out=w1s[:], in_=w1s_d[:])
            cst = sb.tile([P, C_END], F32)
            nc.sync.dma_start(out=cst[:], in_=cst_d[:])
            w23 = sb.tile([P, (NJ * NJ + NJ * NOT) * P], F16)
            nc.sync.dma_start(out=w23[:], in_=w23_d[:])
            f16c = sb.tile([P, P + NOT * N_RULES * P], F16)
            nc.sync.dma_start(out=f16c[:], in_=f16c_d[:])
            ones1 = sb.tile([1, P], F32)
            nc.sync.dma_start(out=ones1[:], in_=ones1_d[:])

            b1t = cst[:, C_B1:C_B1 + NJ]
            b2t = cst[:, C_B2:C_B2 + NJ]
            scl = cst[:, C_SCL:C_SCL + NOT * N_RULES]
            bia = cst[:, C_BIA:C_BIA + NOT * N_RULES]
            eyef = cst[:, C_EYE:C_EYE + P]
            xbp = cst[:, C_XBP:C_END]
            eye16 = f16c[:, :P]
            dgs = f16c[:, P:]

            def W2blk(k, j):
                return w23[:, (k * NJ + j) * P:(k * NJ + j + 1) * P]

            def W3blk(k, j):
                off = NJ * NJ * P
                return w23[:, off + (k * NOT + j) * P:off + (k * NOT + j + 1) * P]

            # ---- P[b] = prod_f x[b,f], replicated across partitions ----
            # P_all[p, t] = P(b = 128*t + p); DMA-permute into one row, then
            # gpsimd partition-broadcast to [128, BL]. No PSUM, no TensorE.
            P_rep = sb.tile([P, BL], F32)
            P_all = sb.tile([P, BL // P], F32)
            nc.vector.tensor_reduce(
                P_all[:],
                xbp.rearrange("p (t f) -> p t f", f=IN_DIM),
                mybir.AxisListType.X, Op.mult,
            )
            P_row = sb.tile([1, BL], F32)
            for t in range(BL // P):
                nc.sync.dma_start(out=P_row[0:1, t * P:(t + 1) * P], in_=P_all[:, t:t + 1])
            nc.gpsimd.partition_broadcast(P_rep[:], P_row[0:1, :])

            def relu_bias(dst, src_psum, bias_col, j):
                # one full-tile op per engine so the two j-tiles run in parallel
                if j % 2 == 0:
                    nc.vector.tensor_scalar(dst[:], src_psum[:], bias_col, 0.0,
                                            Op.add, Op.max)
                else:
                    nc.scalar.activation(dst[:], src_psum[:], RELU,
                                         bias=bias_col, scale=1.0)

            def cs_p(cs):
                return (slice(None), cs)

            hT = []
            h2T = []
            with tc.psum_pool(name="ps_y", bufs=2) as ps_y:
                yT = []
                with tc.psum_pool(name="ps_mlp", bufs=2) as ps_mlp:
                    for j in range(NJ):
                        l1 = ps_mlp.tile([P, BL], F32, tag="mlp")
                        for c in range(NCH):
                            nc.tensor.matmul(
                                l1[:, c * FD:(c + 1) * FD],
                                w1s[:, j * P:(j + 1) * P],
                                xw[:, c * FD:(c + 1) * FD],
                                start=True, stop=True,
                            )
                        h = sb.tile([P, BL], F16, name=f"hT{j}")
                        relu_bias(h[:], l1[:], b1t[:, j:j + 1], j)
                        hT.append(h)
                    for j in range(NJ):
                        l2 = ps_mlp.tile([P, BL], F32, tag="mlp")
                        for c in range(NCH):
                            for k in range(NJ):
                                nc.tensor.matmul(
                                    l2[:, c * FD:(c + 1) * FD],
                                    W2blk(k, j),
                                    hT[k][:, c * FD:(c + 1) * FD],
                                    start=(k == 0), stop=(k == NJ - 1),
                                )
                        h = sb.tile([P, BL], F16, name=f"h2T{j}")
                        relu_bias(h[:], l2[:], b2t[:, j:j + 1], j)
                        h2T.append(h)
                    for j in range(NOT):
                        l3 = ps_y.tile([P, BL], F32, tag="yt")
                        for c in range(NCH):
                            for k in range(NJ):
                                nc.tensor.matmul(
                                    l3[:, c * FD:(c + 1) * FD],
                                    W3blk(k, j),
                                    h2T[k][:, c * FD:(c + 1) * FD],
                                    start=(k == 0), stop=(k == NJ - 1),
                                )
                        yT.append(l3)

                # ---- memberships + D/N + w per o-tile ----
                pad = sb.tile([P, 512], F32, name="pad")
                nc.vector.memset(pad[:, 0:8], 0.0)
                with tc.psum_pool(name="ps_dn", bufs=1) as ps_dn:
                    for ot in range(NOT):
                        D = ps_dn.tile([P, BL], F32, tag="D", name=f"D{ot}")
                        N = ps_dn.tile([P, BL], F32, tag="N", name=f"N{ot}")
                        for r in range(N_RULES):
                            idx = ot * N_RULES + r
                            m = sb.tile([P, BL], F16, tag="m", bufs=4, name=f"m{idx}")
                            if ot == NOT - 1 and r == N_RULES - 1:
                                for c in range(NCH):
                                    cs = slice(c * FD, (c + 1) * FD)
                                    nc.scalar.activation(
                                        m[:, cs], yT[ot][:, cs], DERF,
                                        bias=bia[:, idx:idx + 1], scale=scl[:, idx:idx + 1],
                                    )
                            else:
                                nc.scalar.activation(
                                    m[:], yT[ot][:], DERF,
                                    bias=bia[:, idx:idx + 1], scale=scl[:, idx:idx + 1],
                                )
                            for c in range(NCH):
                                cs = slice(c * FD, (c + 1) * FD)
                                nc.tensor.matmul(D[:, cs], eye16, m[:, cs],
                                                 start=(r == 0), stop=(r == N_RULES - 1))
                                nc.tensor.matmul(N[:, cs], dgs[:, idx * P:(idx + 1) * P], m[:, cs],
                                                 start=(r == 0), stop=(r == N_RULES - 1))
                        rD = sb.tile([P, BL], F32, tag="rD", bufs=2, name=f"rD{ot}")
                        nc.vector.reciprocal_approx_fast(rD[:], D[:])
                        w = sb.tile([P, BL], F32, tag="w", bufs=2, name=f"w{ot}")
                        nc.vector.tensor_tensor(w[:], N[:], rD[:], Op.mult)
                        o = sb.tile([P, BL], F32, tag="osb", bufs=2, name=f"osb{ot}")
                        if ot < NOT - 1:
                            for _ in range(5):
                                nc.vector.tensor_tensor(w[:], w[:], w[:], Op.mult)
                            nc.vector.tensor_tensor(o[:], w[:], P_rep[:], Op.mult)
                            nc.sync.dma_start(out=out_d[ot * P:(ot + 1) * P, :], in_=o[:])
                        else:
                            # last o-tile = serial tail: pipeline halves on DVE + ACT
                            h0 = slice(0, BL // 2)
                            h1 = slice(BL // 2, BL)
                            for _ in range(5):
                                nc.vector.tensor_tensor(w[:, h0], w[:, h0], w[:, h0], Op.mult)
                                nc.scalar.activation(w[:, h1], w[:, h1], SQ)
                            nc.vector.tensor_tensor(o[:, h0], w[:, h0], P_rep[:, h0], Op.mult)
                            nc.sync.dma_start(out=out_d[ot * P:(ot + 1) * P, :BL // 2], in_=o[:, h0])
                            nc.vector.tensor_tensor(o[:, h1], w[:, h1], P_rep[:, h1], Op.mult)
                            nc.sync.dma_start(out=out_d[ot * P:(ot + 1) * P, BL // 2:], in_=o[:, h1])

    nc.finalize()
    _nc_cache = nc
    return nc


def _prepare_in_maps(x, W1, b1, W2, b2, W3, b3, centers, widths, params):
    x = np.ascontiguousarray(x, dtype=np.float32)
    W1 = np.asarray(W1, np.float32); b1 = np.asarray(b1, np.float32)
    W2 = np.asarray(W2, np.float32); b2 = np.asarray(b2, np.float32)
    W3 = np.asarray(W3, np.float32); b3 = np.asarray(b3, np.float32)
    centers = np.asarray(centers, np.float32)
    widths = np.asarray(widths, np.float32)
    params = np.asarray(params, np.float32)

    def pack_w(W, nj_out):
        blocks = []
        for k in range(W.shape[0] // P):
            for j in range(nj_out):
                blocks.append(W[k * P:(k + 1) * P, j * P:(j + 1) * P])
        return np.concatenate(blocks, axis=1)

    w23 = np.ascontiguousarray(
        np.concatenate([pack_w(W2, NJ), pack_w(W3, NOT)], axis=1).astype(np.float16))

    b1t = b1.reshape(NJ, P).T
    b2t = b2.reshape(NJ, P).T
    inv = (1.0 / widths).astype(np.float32)
    biasf = ((b3[:, None] - centers) * inv).astype(np.float32)
    scl = inv.reshape(NOT, P, N_RULES).transpose(1, 0, 2).reshape(P, NOT * N_RULES)
    bia = biasf.reshape(NOT, P, N_RULES).transpose(1, 0, 2).reshape(P, NOT * N_RULES)
    eyef = np.eye(P, dtype=np.float32)

    ph = params.astype(np.float16)
    dgs = np.zeros((P, NOT * N_RULES * P), np.float16)
    for ot in range(NOT):
        for r in range(N_RULES):
            idx = ot * N_RULES + r
            dgs[:, idx * P:(idx + 1) * P] = np.diag(ph[ot * P:(ot + 1) * P, r])
    f16c = np.ascontiguousarray(np.concatenate([np.eye(P, dtype=np.float16), dgs], axis=1))

    ones1 = np.ones((1, P), np.float32)

    # L1 bf16 hi/lo stacking: y1 = W1h.T@xh + W1l.T@xh + W1h.T@xl
    W1h = W1.astype(ml_bf16)
    W1l = (W1 - W1h.astype(np.float32)).astype(ml_bf16)
    w1s = np.ascontiguousarray(np.concatenate([W1h, W1l, W1h], axis=0))  # [96, H]

    in_maps = []
    for i in range(N_CORES):
        xs = x[i * BL:(i + 1) * BL]                              # [BL, 32]
        xT = np.ascontiguousarray(xs.T)                          # [32, BL]
        xh = xT.astype(ml_bf16)
        xl = (xT - xh.astype(np.float32)).astype(ml_bf16)
        xw = np.ascontiguousarray(np.concatenate([xh, xh, xl], axis=0))  # [96, BL]
        xbp = xs.reshape(BL // P, P, IN_DIM).transpose(1, 0, 2).reshape(P, -1)
        cst = np.ascontiguousarray(
            np.concatenate([b1t, b2t, scl, bia, eyef, xbp], axis=1))
        in_maps.append(dict(xw=xw, w1s=w1s, cst=cst, w23=w23, f16c=f16c, ones1=ones1))
    return in_maps


def run(trace=False, **inputs):
    nc = _build()
    in_maps = _prepare_in_maps(**inputs)
    res = run_bass_kernel_spmd(nc, in_maps, core_ids=list(range(N_CORES)), trace=trace)
    outs = [res.results[i]["out"].T for i in range(N_CORES)]     # each [BL, O]
    full = np.ascontiguousarray(np.concatenate(outs, axis=0), dtype=np.float32)
    return full, res


def kernel(**inputs) -> np.ndarray:
    full, _ = run(trace=False, **inputs)
    return full
